# revision 1
# baseline (speedup 1.0000x reference)
"""Two-layer modulated deformable conv (DCNv2) + sync-BN + ReLU for trn2.

Strategy: the data-dependent bilinear sampling / im2col / BN stats are cheap,
regular host work; the two big contractions (einsum 'bckhw,ock->bohw', ~39
GFLOP each) run on 8 NeuronCores via a Bass/Tile matmul kernel, data-parallel
over (batch, HW-half) -> 8 shards.
"""

import numpy as np

B, CIN, H, W = 4, 256, 128, 128
MID, COUT = 128, 256
HW = H * W
K2 = 9
_EPS = 1e-5

_KY = np.array([-1, -1, -1, 0, 0, 0, 1, 1, 1], dtype=np.float32)
_KX = np.array([-1, 0, 1, -1, 0, 1, -1, 0, 1], dtype=np.float32)


# ---------------------------------------------------------------- host pieces
def _im2col(x):
    """x [B,C,H,W] -> cols [B, C*9, H*W] (3x3 SAME, zero pad)."""
    b, c, h, w = x.shape
    xp = np.zeros((b, c, h + 2, w + 2), dtype=x.dtype)
    xp[:, :, 1:-1, 1:-1] = x
    cols = np.empty((b, c, 9, h, w), dtype=x.dtype)
    k = 0
    for dy in range(3):
        for dx in range(3):
            cols[:, :, k] = xp[:, :, dy:dy + h, dx:dx + w]
            k += 1
    return cols.reshape(b, c * 9, h * w)


def _conv3x3_host(cols, w, bias):
    """cols [B, C*9, HW], w [O,C,3,3] -> [B, O, HW]."""
    o = w.shape[0]
    wr = w.reshape(o, -1)
    out = np.matmul(wr[None], cols)  # [B, O, HW]
    return out + bias[None, :, None]


def _bilinear_modulated(x, py, px, mask):
    """x [C,H,W]; py,px,mask [9,H,W] -> modulated samples [C*9, HW]."""
    c, h, w = x.shape
    y0 = np.floor(py)
    x0 = np.floor(px)
    ly = py - y0
    lx = px - x0
    y0i = y0.astype(np.int32)
    x0i = x0.astype(np.int32)
    flat = x.reshape(c, h * w)

    def gather(yi, xi):
        valid = ((yi >= 0) & (yi < h) & (xi >= 0) & (xi < w)).astype(np.float32)
        idx = np.clip(yi, 0, h - 1) * w + np.clip(xi, 0, w - 1)
        v = flat[:, idx.reshape(-1)].reshape(c, *yi.shape)
        return v * valid[None]

    v00 = gather(y0i, x0i)
    v01 = gather(y0i, x0i + 1)
    v10 = gather(y0i + 1, x0i)
    v11 = gather(y0i + 1, x0i + 1)
    w00 = ((1 - ly) * (1 - lx) * mask)[None]
    w01 = ((1 - ly) * lx * mask)[None]
    w10 = (ly * (1 - lx) * mask)[None]
    w11 = (ly * lx * mask)[None]
    s = v00 * w00 + v01 * w01 + v10 * w10 + v11 * w11  # [C,9,H,W]
    return s.reshape(c * 9, h * w).astype(np.float32)


def _sampled_for_layer(x, w_off, b_off):
    """x [B,C,H,W] -> modulated sampled cols [B, C*9, HW]."""
    b, c, h, w = x.shape
    om = _conv3x3_host(_im2col(x), w_off, b_off).reshape(b, 27, h, w)
    off_y = om[:, :K2]
    off_x = om[:, K2:2 * K2]
    mask = 1.0 / (1.0 + np.exp(-om[:, 2 * K2:]))
    yy = np.arange(h, dtype=np.float32)
    xx = np.arange(w, dtype=np.float32)
    py = yy[None, None, :, None] + _KY[None, :, None, None] + off_y  # [B,9,H,W]
    px = xx[None, None, None, :] + _KX[None, :, None, None] + off_x
    out = np.empty((b, c * 9, h * w), dtype=np.float32)
    for i in range(b):
        out[i] = _bilinear_modulated(x[i], py[i], px[i], mask[i])
    return out


def _bn_relu(x, gamma, beta):
    """x [B,O,HW] -> same, sync-BN (biased var) + affine + relu."""
    mu = x.mean(axis=(0, 2), keepdims=True)
    var = ((x - mu) ** 2).mean(axis=(0, 2), keepdims=True)
    y = (x - mu) / np.sqrt(var + _EPS)
    y = y * gamma[None, :, None] + beta[None, :, None]
    return np.maximum(y, 0.0)


# ---------------------------------------------------------------- bass kernel
_NT = 512  # fp32 moving-operand max free dim


def _build_matmul_nc(kdim, odim, ncols):
    """out[odim, ncols] = lhsT.T @ rhs, lhsT [kdim, odim], rhs [kdim, ncols].

    Raw-bass double-buffered pipeline: sync streams [kdim, 512] rhs slabs,
    PE runs nk-deep PSUM accumulation groups (one standalone wait per slab),
    DVE evicts PSUM -> SBUF, sync stores. Buffer-reuse safety is chained
    through the dve/store sems so no instruction needs >1 wait.
    """
    from contextlib import ExitStack

    import concourse.bass as bass
    import concourse.mybir as mybir

    f32 = mybir.dt.float32
    nc = bass.Bass()
    rhs = nc.dram_tensor("rhs", [kdim, ncols], f32, kind="ExternalInput")
    lhsT = nc.dram_tensor("lhsT", [kdim, odim], f32, kind="ExternalInput")
    out = nc.dram_tensor("out", [odim, ncols], f32, kind="ExternalOutput")
    nk, nm, nn = kdim // 128, odim // 128, ncols // _NT
    nps = 2 * nm  # psum/out ring depth

    rhs_r = rhs.rearrange("(k p) w -> p k w", p=128)    # [128, nk, ncols]
    lhsT_r = lhsT.rearrange("(k p) o -> p k o", p=128)  # [128, nk, odim]

    with ExitStack() as es:
        wtile = es.enter_context(nc.sbuf_tensor("wtile", [128, nk * odim], f32))
        rbufs = [es.enter_context(nc.sbuf_tensor(f"rbuf{i}", [128, nk * _NT], f32))
                 for i in range(2)]
        obufs = [es.enter_context(nc.sbuf_tensor(f"obuf{i}", [128, _NT], f32))
                 for i in range(nps)]
        psums = [es.enter_context(nc.psum_tensor(f"psum{i}", [128, _NT], f32))
                 for i in range(nps)]
        rd = es.enter_context(nc.semaphore())
        pe = es.enter_context(nc.semaphore())
        dve = es.enter_context(nc.semaphore())
        st = es.enter_context(nc.semaphore())
        block = es.enter_context(nc.Block())

        def slab(n):
            return rhs_r[:, :, n * _NT:(n + 1) * _NT]

        def rb3(i):
            return rbufs[i][:].rearrange("p (k w) -> p k w", k=nk)

        @block.sync
        def _(sync):
            sync.dma_start(
                wtile[:].rearrange("p (k o) -> p k o", k=nk),
                lhsT_r[:, :, :]).then_inc(rd, 16)
            for pre in range(min(2, nn)):
                sync.dma_start(rb3(pre), slab(pre)).then_inc(rd, 16)
            for n in range(nn):
                for m in range(nm):
                    g = n * nm + m
                    sync.wait_ge(dve, g + 1)
                    sync.dma_start(
                        out[m * 128:(m + 1) * 128, n * _NT:(n + 1) * _NT],
                        obufs[g % nps][:]).then_inc(st, 16)
                if n + 2 < nn:
                    sync.dma_start(rb3((n + 2) % 2),
                                   slab(n + 2)).then_inc(rd, 16)
            sync.wait_ge(st, 16 * nn * nm)

        @block.tensor
        def _(tensor):
            for n in range(nn):
                tensor.wait_ge(rd, 16 * (n + 2))
                for m in range(nm):
                    g = n * nm + m
                    ps = psums[g % nps]
                    mm = None
                    for k in range(nk):
                        mm = tensor.matmul(
                            ps[:],
                            wtile[:, k * odim + m * 128:
                                  k * odim + (m + 1) * 128],
                            rbufs[n % 2][:, k * _NT:(k + 1) * _NT],
                            start=(k == 0), stop=(k == nk - 1))
                    mm.then_inc(pe, 1)

        @block.vector
        def _(vector):
            for n in range(nn):
                for m in range(nm):
                    g = n * nm + m
                    vector.wait_ge(pe, g + 1)
                    if g >= nps:
                        vector.wait_ge(st, 16 * (g + 1 - nps))
                    vector.tensor_copy(
                        obufs[g % nps][:], psums[g % nps][:]).then_inc(dve, 1)
    return nc


_NC_CACHE = {}
DEVICE_STATS = []  # one entry per device invocation: {wall_ns, exec_time_ns}


def _device_contract(sampled, wr):
    """sampled [B, K, HW], wr [O, K] -> [B, O, HW] on 8 cores (b, hw-half)."""
    import time

    from concourse import bass_utils

    bdim, kdim, hw = sampled.shape
    odim = wr.shape[0]
    half = hw // 2
    key = (kdim, odim, half)
    if key not in _NC_CACHE:
        _NC_CACHE[key] = _build_matmul_nc(kdim, odim, half)
    nc = _NC_CACHE[key]
    lhsT = np.ascontiguousarray(wr.T)  # [K, O]
    in_maps = []
    for s in range(8):
        b, hh = s // 2, s % 2
        in_maps.append({
            "rhs": np.ascontiguousarray(sampled[b, :, hh * half:(hh + 1) * half]),
            "lhsT": lhsT,
        })
    t0 = time.perf_counter_ns()
    res = bass_utils.run_bass_kernel_spmd(nc, in_maps, core_ids=list(range(8)))
    t1 = time.perf_counter_ns()
    DEVICE_STATS.append({"wall_ns": t1 - t0,
                         "exec_time_ns": res.exec_time_ns})
    out = np.empty((bdim, odim, hw), dtype=np.float32)
    for s in range(8):
        b, hh = s // 2, s % 2
        out[b, :, hh * half:(hh + 1) * half] = res.results[s]["out"]
    return out


def _contract(sampled, wr):
    try:
        return _device_contract(sampled, wr)
    except Exception as e:  # pragma: no cover - device fallback
        import traceback
        traceback.print_exc()
        print(f"[kernel] device path failed ({e!r}); numpy fallback")
        return np.matmul(wr[None], sampled)


# ---------------------------------------------------------------- entry point
def kernel(x, w_off1, b_off1, w1, b1, g1, be1,
           w_off2, b_off2, w2, b2, g2, be2):
    x = np.asarray(x, dtype=np.float32)

    s1 = _sampled_for_layer(x, np.asarray(w_off1), np.asarray(b_off1))
    y1 = _contract(s1, np.asarray(w1).reshape(MID, -1))
    y1 += np.asarray(b1)[None, :, None]
    h1 = _bn_relu(y1, np.asarray(g1), np.asarray(be1)).reshape(B, MID, H, W)

    s2 = _sampled_for_layer(h1, np.asarray(w_off2), np.asarray(b_off2))
    y2 = _contract(s2, np.asarray(w2).reshape(COUT, -1))
    y2 += np.asarray(b2)[None, :, None]
    h2 = _bn_relu(y2, np.asarray(g2), np.asarray(be2)).reshape(B, COUT, H, W)
    return h2

